# revision 1
# baseline (speedup 1.0000x reference)
"""Causal self-attention (B=2, L=2048, E=2048, H=16, D=128) on 8 trn2 cores.

Sharding: Megatron-style tensor parallel over heads. Each core owns 2 heads:
Wq/Wk/Wv column-split, Wo row-split; x replicated (pre-transposed, bf16).
Each core computes a partial output projection; host sums partials + bias.

Device kernel per core, per (batch, head):
  - qT/kT/vT [D=128, L] built with weight slices as the stationary matmul
    operand (xT streams).  Host permutes Wq/Wk columns to (evens, odds) order
    and stacks q/k halves so RoPE is 6 contiguous [128,512] DVE ops.
  - scores per 128-row q-tile over the causal band, fp32 PSUM; diagonal block
    gets an additive -1e30 triangular mask; Exp on ACT with accum_out giving
    row sums; P normalized by reciprocal(rowsum); transposed [128,band] ->
    [128, nb, 128] with one xbar DMA-transpose; attn@v and out-projection
    accumulate in PSUM (out-proj sums the core's 2 heads).
"""

import os

import numpy as np
import ml_dtypes

import concourse.bass as bass
import concourse.tile as tile
from concourse import bacc, mybir
from concourse.bass_utils import run_bass_kernel_spmd

BF16 = mybir.dt.bfloat16
F32 = mybir.dt.float32
AF = mybir.ActivationFunctionType
ALU = mybir.AluOpType

B, L, E = 2, 2048, 2048
H, D = 16, 128
NCORES = 8
HPC = H // NCORES          # heads per core
KT = E // 128              # 16 contraction tiles
LC = L // 512              # 4 column chunks of L
QT = L // 128              # 16 q tiles
THETA = 10000.0

_PROG = None


def _build_program():
    nc = bacc.Bacc("TRN2", target_bir_lowering=False, debug=False,
                   enable_asserts=False)

    xT_d = nc.dram_tensor("xT", [B, E, L], BF16, kind="ExternalInput").ap()
    w_d = nc.dram_tensor("w", [E, HPC * 3 * 128], BF16, kind="ExternalInput").ap()
    wo_d = nc.dram_tensor("wo", [HPC, D, E], BF16, kind="ExternalInput").ap()
    cos_d = nc.dram_tensor("cosf", [128, L], F32, kind="ExternalInput").ap()
    sin_d = nc.dram_tensor("sinf", [128, L], F32, kind="ExternalInput").ap()
    tri_d = nc.dram_tensor("tri", [128, 128], F32, kind="ExternalInput").ap()
    y_d = nc.dram_tensor("y", [B, L, E], BF16, kind="ExternalOutput").ap()

    with tile.TileContext(nc) as tc:
        with tc.tile_pool(name="consts", bufs=1) as cpool, \
             tc.tile_pool(name="xt", bufs=8) as xpool, \
             tc.tile_pool(name="rope", bufs=8) as rpool, \
             tc.tile_pool(name="qkv", bufs=2) as qkvpool, \
             tc.tile_pool(name="pp", bufs=3) as ppool, \
             tc.tile_pool(name="small", bufs=8) as spool, \
             tc.tile_pool(name="outp", bufs=12) as opool, \
             tc.tile_pool(name="ys", bufs=2) as ypool, \
             tc.tile_pool(name="ps512", bufs=6, space="PSUM") as ps512, \
             tc.tile_pool(name="psy", bufs=2, space="PSUM") as psy:

            w_sb = cpool.tile([128, KT, HPC, 3, 128], BF16, tag="w")
            nc.sync.dma_start(w_sb[:], w_d.rearrange("(kt p) c -> p kt c", p=128))
            wo_sb = cpool.tile([128, HPC, E], BF16, tag="wo")
            nc.sync.dma_start(wo_sb[:], wo_d.rearrange("h p e -> p h e"))
            cosf = cpool.tile([128, L], F32, tag="cos")
            nc.sync.dma_start(cosf[:], cos_d[:])
            sinf = cpool.tile([128, L], F32, tag="sin")
            nc.sync.dma_start(sinf[:], sin_d[:])
            tri = cpool.tile([128, 128], F32, tag="tri")
            nc.sync.dma_start(tri[:], tri_d[:])

            for rep in range(int(os.environ.get("KREP", "1"))):
              for b in range(B):
                outT = [None] * HPC
                for h in range(HPC):
                    # ---- QKV projection + RoPE ----
                    qT = qkvpool.tile([128, L], BF16, tag="qT")
                    kTt = qkvpool.tile([128, L], BF16, tag="kT")
                    vTs = qkvpool.tile([128, L], BF16, tag="vT")
                    for lc in range(LC):
                        ls = lc * 512
                        pA = ps512.tile([128, 512], F32, tag="ps512")
                        pB = ps512.tile([128, 512], F32, tag="ps512")
                        pV = ps512.tile([128, 512], F32, tag="ps512")
                        for kt in range(KT):
                            xt = xpool.tile([128, 512], BF16, tag="xt")
                            nc.sync.dma_start(
                                xt[:], xT_d[b, kt * 128:(kt + 1) * 128, ls:ls + 512])
                            st = kt == 0
                            sp = kt == KT - 1
                            nc.tensor.matmul(pA[:], w_sb[:, kt, h, 0, :], xt[:],
                                             start=st, stop=sp)
                            nc.tensor.matmul(pB[:], w_sb[:, kt, h, 1, :], xt[:],
                                             start=st, stop=sp)
                            nc.tensor.matmul(pV[:], w_sb[:, kt, h, 2, :], xt[:],
                                             start=st, stop=sp)
                        # RoPE: A = [x1q;x1k], B = [x2q;x2k]
                        t1 = rpool.tile([128, 512], F32, tag="rt")
                        nc.vector.tensor_mul(t1[:], pA[:], cosf[:, ls:ls + 512])
                        t2 = rpool.tile([128, 512], F32, tag="rt")
                        nc.vector.tensor_mul(t2[:], pB[:], sinf[:, ls:ls + 512])
                        et = rpool.tile([128, 512], BF16, tag="ro")
                        nc.gpsimd.tensor_sub(et[:], t1[:], t2[:])
                        t3 = rpool.tile([128, 512], F32, tag="rt")
                        nc.vector.tensor_mul(t3[:], pA[:], sinf[:, ls:ls + 512])
                        t4 = rpool.tile([128, 512], F32, tag="rt")
                        nc.vector.tensor_mul(t4[:], pB[:], cosf[:, ls:ls + 512])
                        ot = rpool.tile([128, 512], BF16, tag="ro")
                        nc.gpsimd.tensor_add(ot[:], t3[:], t4[:])
                        nc.vector.tensor_copy(qT[0:64, ls:ls + 512], et[0:64, :])
                        nc.vector.tensor_copy(qT[64:128, ls:ls + 512], ot[0:64, :])
                        nc.vector.tensor_copy(kTt[0:64, ls:ls + 512], et[64:128, :])
                        nc.vector.tensor_copy(kTt[64:128, ls:ls + 512], ot[64:128, :])
                        nc.scalar.copy(vTs[:, ls:ls + 512], pV[:])
                    v_nat = qkvpool.tile([128, KT, 128], BF16, tag="vn")
                    nc.scalar.dma_start_transpose(out=v_nat[:], in_=vTs[:])

                    # ---- attention ----
                    outT[h] = [
                        opool.tile([128, 512], BF16, tag="outT",
                                   name=f"outT_b{b}h{h}g{g}")
                        for g in range(4)
                    ]
                    po = None
                    for i in range(QT):
                        band = (i + 1) * 128
                        nch = (band + 511) // 512
                        pt_t = ppool.tile([128, L], BF16, tag="P")
                        ds = spool.tile([128, 4], F32, tag="ds")
                        for c in range(nch):
                            c0 = c * 512
                            w = min(512, band - c0)
                            s_ps = ps512.tile([128, 512], F32, tag="ps512")
                            nc.tensor.matmul(
                                s_ps[:, 0:w], qT[:, i * 128:(i + 1) * 128],
                                kTt[:, c0:c0 + w], start=True, stop=True)
                            if c == nch - 1:
                                nc.vector.tensor_tensor(
                                    s_ps[:, w - 128:w], s_ps[:, w - 128:w],
                                    tri[:], op=ALU.add)
                            nc.scalar.activation(
                                pt_t[:, c0:c0 + w], s_ps[:, 0:w], AF.Exp,
                                accum_out=ds[:, c:c + 1])
                        dtot = spool.tile([128, 1], F32, tag="dt")
                        if nch > 1:
                            nc.vector.reduce_sum(dtot[:], ds[:, 0:nch],
                                                 axis=mybir.AxisListType.X)
                        else:
                            nc.vector.tensor_copy(dtot[:], ds[:, 0:1])
                        rinv = spool.tile([128, 1], F32, tag="ri")
                        nc.vector.reciprocal(rinv[:], dtot[:])
                        nc.vector.tensor_scalar_mul(pt_t[:, 0:band], pt_t[:, 0:band],
                                                    rinv[:])
                        ptr = ppool.tile([128, KT, 128], BF16, tag="PT")
                        nc.scalar.dma_start_transpose(out=ptr[:, 0:i + 1, :],
                                                      in_=pt_t[:, 0:band])
                        if i % 4 == 0:
                            po = ps512.tile([128, 512], F32, tag="ps512")
                        osl = (i % 4) * 128
                        for kb in range(i + 1):
                            nc.tensor.matmul(
                                po[:, osl:osl + 128], v_nat[:, kb, :],
                                ptr[:, kb, :], start=(kb == 0), stop=(kb == i))
                        if i % 4 == 3:
                            nc.scalar.copy(outT[h][i // 4][:], po[:])

                # ---- output projection (sums the core's heads) ----
                for i in range(QT):
                    ysb = ypool.tile([128, E], BF16, tag="ysb")
                    qs = (i % 4) * 128
                    for ec in range(4):
                        es = ec * 512
                        yp = psy.tile([128, 512], F32, tag="psy")
                        for h in range(HPC):
                            nc.tensor.matmul(
                                yp[:], outT[h][i // 4][:, qs:qs + 128],
                                wo_sb[:, h, es:es + 512],
                                start=(h == 0), stop=(h == HPC - 1))
                        if ec % 2 == 0:
                            nc.scalar.copy(ysb[:, es:es + 512], yp[:])
                        else:
                            nc.vector.tensor_copy(ysb[:, es:es + 512], yp[:])
                    nc.scalar.dma_start(y_d[b, i * 128:(i + 1) * 128, :], ysb[:])

    nc.compile()
    return nc


def _get_program():
    global _PROG
    if _PROG is None:
        _PROG = _build_program()
    return _PROG


def make_in_maps(x, Wq, Wk, Wv, Wo):
    """Host-side sharding/layout prep. Returns list of 8 per-core input maps."""
    bf = ml_dtypes.bfloat16
    x = np.asarray(x, np.float32)
    Wq = np.asarray(Wq, np.float32)
    Wk = np.asarray(Wk, np.float32)
    Wv = np.asarray(Wv, np.float32)
    Wo = np.asarray(Wo, np.float32)

    xT = np.ascontiguousarray(x.transpose(0, 2, 1)).astype(bf)  # [B, E, L]

    inv = THETA ** (-np.arange(0, D, 2, dtype=np.float32) / D)  # [64]
    ang = np.arange(L, dtype=np.float32)[:, None] * inv[None, :]  # [L, 64]
    cosf = np.ascontiguousarray(np.concatenate([np.cos(ang).T] * 2, axis=0)
                                ).astype(np.float32)  # [128, L]
    sinf = np.ascontiguousarray(np.concatenate([np.sin(ang).T] * 2, axis=0)
                                ).astype(np.float32)
    r = np.arange(128)
    tri = np.where(r[None, :] <= r[:, None], 0.0, -1e30).astype(np.float32)

    qscale = np.float32(D ** -0.5)
    ev = np.arange(0, D, 2)
    od = np.arange(1, D, 2)

    maps = []
    for core in range(NCORES):
        w_all = np.empty((E, HPC, 3, 128), np.float32)
        for h in range(HPC):
            g = core * HPC + h
            c0 = g * D
            w_all[:, h, 0, 0:64] = Wq[:, c0 + ev] * qscale
            w_all[:, h, 0, 64:128] = Wk[:, c0 + ev]
            w_all[:, h, 1, 0:64] = Wq[:, c0 + od] * qscale
            w_all[:, h, 1, 64:128] = Wk[:, c0 + od]
            w_all[:, h, 2, :] = Wv[:, c0:c0 + D]
        wo_c = Wo[core * HPC * D:(core + 1) * HPC * D, :].reshape(HPC, D, E)
        maps.append({
            "xT": xT,
            "w": np.ascontiguousarray(w_all.reshape(E, HPC * 3 * 128)).astype(bf),
            "wo": np.ascontiguousarray(wo_c).astype(bf),
            "cosf": cosf,
            "sinf": sinf,
            "tri": tri,
        })
    return maps


def kernel(x, Wq, Wk, Wv, Wo, bo):
    nc = _get_program()
    maps = make_in_maps(x, Wq, Wk, Wv, Wo)
    res = run_bass_kernel_spmd(nc, maps, core_ids=list(range(NCORES)))
    y = np.zeros((B, L, E), np.float64)
    for c in range(NCORES):
        y += np.asarray(res.results[c]["y"], np.float64)
    y += np.asarray(bo, np.float64)[None, None, :]
    return y.astype(np.float32)



# revision 2
# speedup vs baseline: 3.0579x; 3.0579x over previous
"""Causal self-attention (B=2, L=2048, E=2048, H=16, D=128) on ONE trn2 core.

Measured bottleneck in this harness is the per-call axon/PJRT dispatch path,
not device compute: with an 8-core mesh every call re-shards ~300MB of
replicated inputs/partial outputs through the tunnel (~25ms/iter), while a
single-core program keeps all buffers device-resident (bytes are free) and
pays only the ~10ms fixed dispatch. Device exec (~4-5ms, all 16 heads x 2
batches) pipelines under that. So: one core, whole problem, minimal I/O.

Device kernel per batch:
  - xT[b] resident in SBUF [128, KT, L] bf16; per-head weight slices streamed
    (stationary operand), so QKV needs no x re-DMA per head.
  - per head: qT/kT/vT [D=128, L] via matmul; host pre-permutes Wq/Wk columns
    to (evens, odds) so RoPE is 6 contiguous [128,512] elementwise ops.
  - scores per 128-row q-tile over the causal band, fp32 PSUM; additive -1e30
    triangular mask on the diagonal block; Exp on ACT with accum_out row sums;
    P normalized by reciprocal(rowsum); transposed with one xbar DMA;
    attn@v accumulates in PSUM; per-head output tiles parked in DRAM scratch.
  - out-projection: ec-outer over 512-wide E slices, Wo slice resident,
    attention outputs streamed back, PSUM accumulates over all 16 heads.
Host adds bo and casts to f32.
"""

import os

import numpy as np
import ml_dtypes

import concourse.bass as bass
import concourse.tile as tile
from concourse import bacc, mybir
from concourse.bass_utils import run_bass_kernel_spmd

BF16 = mybir.dt.bfloat16
F32 = mybir.dt.float32
AF = mybir.ActivationFunctionType
ALU = mybir.AluOpType

B, L, E = 2, 2048, 2048
H, D = 16, 128
NCORES = 1
KT = E // 128              # 16 contraction tiles
LC = L // 512              # 4 column chunks of L
QT = L // 128              # 16 q tiles
EC = E // 512              # 4 output-column chunks
THETA = 10000.0

_PROG = None


def _build_program():
    nc = bacc.Bacc("TRN2", target_bir_lowering=False, debug=False,
                   enable_asserts=False)

    xT_d = nc.dram_tensor("xT", [B, E, L], BF16, kind="ExternalInput").ap()
    w_d = nc.dram_tensor("w", [H, E, 3 * 128], BF16, kind="ExternalInput").ap()
    wo_d = nc.dram_tensor("wo", [H, D, E], BF16, kind="ExternalInput").ap()
    cos_d = nc.dram_tensor("cosf", [128, L], F32, kind="ExternalInput").ap()
    sin_d = nc.dram_tensor("sinf", [128, L], F32, kind="ExternalInput").ap()
    tri_d = nc.dram_tensor("tri", [128, 128], F32, kind="ExternalInput").ap()
    aT_d = nc.dram_tensor("aT", [H, 128, L], BF16, kind="Internal").ap()
    y_d = nc.dram_tensor("y", [B, L, E], BF16, kind="ExternalOutput").ap()

    with tile.TileContext(nc) as tc:
        with tc.tile_pool(name="consts", bufs=1) as cpool, \
             tc.tile_pool(name="xres", bufs=1) as xpool, \
             tc.tile_pool(name="wstr", bufs=2) as wpool, \
             tc.tile_pool(name="rope", bufs=4) as rpool, \
             tc.tile_pool(name="qkv", bufs=1) as qkvpool, \
             tc.tile_pool(name="pp", bufs=2) as ppool, \
             tc.tile_pool(name="small", bufs=8) as spool, \
             tc.tile_pool(name="outp", bufs=4) as opool, \
             tc.tile_pool(name="oproj", bufs=2) as projpool, \
             tc.tile_pool(name="ys", bufs=2) as ypool, \
             tc.tile_pool(name="ps512", bufs=6, space="PSUM") as ps512, \
             tc.tile_pool(name="psy", bufs=2, space="PSUM") as psy:

            cosf = cpool.tile([128, L], F32, tag="cos")
            nc.sync.dma_start(cosf[:], cos_d[:])
            sinf = cpool.tile([128, L], F32, tag="sin")
            nc.sync.dma_start(sinf[:], sin_d[:])
            tri = cpool.tile([128, 128], F32, tag="tri")
            nc.sync.dma_start(tri[:], tri_d[:])

            for rep in range(int(os.environ.get("KREP", "1"))):
              for b in range(B):
                xsb = xpool.tile([128, KT, L], BF16, tag="xsb")
                nc.sync.dma_start(
                    xsb[:], xT_d[b].rearrange("(kt p) l -> p kt l", p=128))

                for h in range(H):
                    wsb = wpool.tile([128, KT, 3 * 128], BF16, tag="wsb")
                    nc.sync.dma_start(
                        wsb[:], w_d[h].rearrange("(kt p) c -> p kt c", p=128))

                    # ---- QKV projection + RoPE ----
                    qT = qkvpool.tile([128, L], BF16, tag="qT")
                    kTt = qkvpool.tile([128, L], BF16, tag="kT")
                    vTs = qkvpool.tile([128, L], BF16, tag="vT")
                    for lc in range(LC):
                        ls = lc * 512
                        pA = ps512.tile([128, 512], F32, tag="ps512")
                        pB = ps512.tile([128, 512], F32, tag="ps512")
                        pV = ps512.tile([128, 512], F32, tag="ps512")
                        for kt in range(KT):
                            st = kt == 0
                            sp = kt == KT - 1
                            nc.tensor.matmul(pA[:], wsb[:, kt, 0:128],
                                             xsb[:, kt, ls:ls + 512],
                                             start=st, stop=sp)
                            nc.tensor.matmul(pB[:], wsb[:, kt, 128:256],
                                             xsb[:, kt, ls:ls + 512],
                                             start=st, stop=sp)
                            nc.tensor.matmul(pV[:], wsb[:, kt, 256:384],
                                             xsb[:, kt, ls:ls + 512],
                                             start=st, stop=sp)
                        # RoPE: A = [x1q;x1k], B = [x2q;x2k]
                        t1 = rpool.tile([128, 512], F32, tag="rt")
                        nc.vector.tensor_mul(t1[:], pA[:], cosf[:, ls:ls + 512])
                        t2 = rpool.tile([128, 512], F32, tag="rt")
                        nc.vector.tensor_mul(t2[:], pB[:], sinf[:, ls:ls + 512])
                        et = rpool.tile([128, 512], BF16, tag="ro")
                        nc.gpsimd.tensor_sub(et[:], t1[:], t2[:])
                        t3 = rpool.tile([128, 512], F32, tag="rt")
                        nc.vector.tensor_mul(t3[:], pA[:], sinf[:, ls:ls + 512])
                        t4 = rpool.tile([128, 512], F32, tag="rt")
                        nc.vector.tensor_mul(t4[:], pB[:], cosf[:, ls:ls + 512])
                        ot = rpool.tile([128, 512], BF16, tag="ro")
                        nc.gpsimd.tensor_add(ot[:], t3[:], t4[:])
                        nc.vector.tensor_copy(qT[0:64, ls:ls + 512], et[0:64, :])
                        nc.vector.tensor_copy(qT[64:128, ls:ls + 512], ot[0:64, :])
                        nc.vector.tensor_copy(kTt[0:64, ls:ls + 512], et[64:128, :])
                        nc.vector.tensor_copy(kTt[64:128, ls:ls + 512], ot[64:128, :])
                        nc.scalar.copy(vTs[:, ls:ls + 512], pV[:])
                    v_nat = qkvpool.tile([128, KT, 128], BF16, tag="vn")
                    nc.scalar.dma_start_transpose(out=v_nat[:], in_=vTs[:])

                    # ---- attention ----
                    po = None
                    for i in range(QT):
                        band = (i + 1) * 128
                        nch = (band + 511) // 512
                        pt_t = ppool.tile([128, L], BF16, tag="P")
                        ds = spool.tile([128, 4], F32, tag="ds")
                        for c in range(nch):
                            c0 = c * 512
                            w = min(512, band - c0)
                            s_ps = ps512.tile([128, 512], F32, tag="ps512")
                            nc.tensor.matmul(
                                s_ps[:, 0:w], qT[:, i * 128:(i + 1) * 128],
                                kTt[:, c0:c0 + w], start=True, stop=True)
                            if c == nch - 1:
                                nc.vector.tensor_tensor(
                                    s_ps[:, w - 128:w], s_ps[:, w - 128:w],
                                    tri[:], op=ALU.add)
                            nc.scalar.activation(
                                pt_t[:, c0:c0 + w], s_ps[:, 0:w], AF.Exp,
                                accum_out=ds[:, c:c + 1])
                        dtot = spool.tile([128, 1], F32, tag="dt")
                        if nch > 1:
                            nc.vector.reduce_sum(dtot[:], ds[:, 0:nch],
                                                 axis=mybir.AxisListType.X)
                        else:
                            nc.vector.tensor_copy(dtot[:], ds[:, 0:1])
                        rinv = spool.tile([128, 1], F32, tag="ri")
                        nc.vector.reciprocal(rinv[:], dtot[:])
                        nc.vector.tensor_scalar_mul(pt_t[:, 0:band], pt_t[:, 0:band],
                                                    rinv[:])
                        ptr = ppool.tile([128, KT, 128], BF16, tag="PT")
                        nc.scalar.dma_start_transpose(out=ptr[:, 0:i + 1, :],
                                                      in_=pt_t[:, 0:band])
                        if i % 4 == 0:
                            po = ps512.tile([128, 512], F32, tag="ps512")
                        osl = (i % 4) * 128
                        for kb in range(i + 1):
                            nc.tensor.matmul(
                                po[:, osl:osl + 128], v_nat[:, kb, :],
                                ptr[:, kb, :], start=(kb == 0), stop=(kb == i))
                        if i % 4 == 3:
                            g = i // 4
                            outsb = opool.tile([128, 512], BF16, tag="outT")
                            nc.scalar.copy(outsb[:], po[:])
                            nc.scalar.dma_start(
                                aT_d[h, :, g * 512:(g + 1) * 512], outsb[:])

                # ---- output projection (sums all 16 heads in PSUM) ----
                for ec in range(EC):
                    es = ec * 512
                    wo_ec = projpool.tile([128, H, 512], BF16, tag="wo")
                    nc.sync.dma_start(
                        wo_ec[:], wo_d[:, :, es:es + 512].rearrange("h p e -> p h e"))
                    for i in range(QT):
                        asb = projpool.tile([128, H, 128], BF16, tag="asb")
                        nc.sync.dma_start(
                            asb[:],
                            aT_d[:, :, i * 128:(i + 1) * 128].rearrange(
                                "h p c -> p h c"))
                        yp = psy.tile([128, 512], F32, tag="psy")
                        for hh in range(H):
                            nc.tensor.matmul(
                                yp[:], asb[:, hh, :], wo_ec[:, hh, :],
                                start=(hh == 0), stop=(hh == H - 1))
                        ysb = ypool.tile([128, 512], BF16, tag="ysb")
                        if i % 2 == 0:
                            nc.scalar.copy(ysb[:], yp[:])
                        else:
                            nc.vector.tensor_copy(ysb[:], yp[:])
                        nc.scalar.dma_start(
                            y_d[b, i * 128:(i + 1) * 128, es:es + 512], ysb[:])

    nc.compile()
    return nc


def _get_program():
    global _PROG
    if _PROG is None:
        _PROG = _build_program()
    return _PROG


def make_in_maps(x, Wq, Wk, Wv, Wo):
    """Host-side layout prep. Returns list of NCORES (=1) input maps."""
    bf = ml_dtypes.bfloat16
    x = np.asarray(x, np.float32)
    Wq = np.asarray(Wq, np.float32)
    Wk = np.asarray(Wk, np.float32)
    Wv = np.asarray(Wv, np.float32)
    Wo = np.asarray(Wo, np.float32)

    xT = np.ascontiguousarray(x.transpose(0, 2, 1)).astype(bf)  # [B, E, L]

    inv = THETA ** (-np.arange(0, D, 2, dtype=np.float32) / D)  # [64]
    ang = np.arange(L, dtype=np.float32)[:, None] * inv[None, :]  # [L, 64]
    cosf = np.ascontiguousarray(np.concatenate([np.cos(ang).T] * 2, axis=0)
                                ).astype(np.float32)  # [128, L]
    sinf = np.ascontiguousarray(np.concatenate([np.sin(ang).T] * 2, axis=0)
                                ).astype(np.float32)
    r = np.arange(128)
    tri = np.where(r[None, :] <= r[:, None], 0.0, -1e30).astype(np.float32)

    qscale = np.float32(D ** -0.5)
    ev = np.arange(0, D, 2)
    od = np.arange(1, D, 2)

    w_all = np.empty((H, E, 3, 128), np.float32)
    for h in range(H):
        c0 = h * D
        w_all[h, :, 0, 0:64] = Wq[:, c0 + ev] * qscale
        w_all[h, :, 0, 64:128] = Wk[:, c0 + ev]
        w_all[h, :, 1, 0:64] = Wq[:, c0 + od] * qscale
        w_all[h, :, 1, 64:128] = Wk[:, c0 + od]
        w_all[h, :, 2, :] = Wv[:, c0:c0 + D]

    return [{
        "xT": xT,
        "w": np.ascontiguousarray(w_all.reshape(H, E, 3 * 128)).astype(bf),
        "wo": np.ascontiguousarray(Wo.reshape(H, D, E)).astype(bf),
        "cosf": cosf,
        "sinf": sinf,
        "tri": tri,
    }]


def kernel(x, Wq, Wk, Wv, Wo, bo):
    nc = _get_program()
    maps = make_in_maps(x, Wq, Wk, Wv, Wo)
    res = run_bass_kernel_spmd(nc, maps, core_ids=list(range(NCORES)))
    y = np.asarray(res.results[0]["y"], np.float64)
    y = y + np.asarray(bo, np.float64)[None, None, :]
    return y.astype(np.float32)


# revision 12
# speedup vs baseline: 3.2981x; 1.0785x over previous
"""Causal self-attention (B=2, L=2048, E=2048, H=16, D=128) on ONE trn2 core.

Measured bottleneck in this harness is the per-call axon/PJRT dispatch path,
not device compute: with an 8-core mesh every call re-shards ~300MB of
replicated inputs/partial outputs through the tunnel (~25ms/iter), while a
single-core program keeps all buffers device-resident (bytes are free) and
pays only the ~10ms fixed dispatch. Device exec (~4-5ms, all 16 heads x 2
batches) pipelines under that. So: one core, whole problem, minimal I/O.

Device kernel per batch:
  - xT[b] resident in SBUF [128, KT, L] bf16; per-head weight slices streamed
    (stationary operand), so QKV needs no x re-DMA per head.
  - per head: qT/kT/vT [D=128, L] via matmul; host pre-permutes Wq/Wk columns
    to (evens, odds) so RoPE is 6 contiguous [128,512] elementwise ops.
  - scores per 128-row q-tile over the causal band, fp32 PSUM; additive -1e30
    triangular mask on the diagonal block; Exp on ACT with accum_out row sums;
    P normalized by reciprocal(rowsum); transposed with one xbar DMA;
    attn@v accumulates in PSUM; per-head output tiles parked in DRAM scratch.
  - out-projection: ec-outer over 512-wide E slices, Wo slice resident,
    attention outputs streamed back, PSUM accumulates over all 16 heads.
Host adds bo and casts to f32.
"""

import os

import numpy as np
import ml_dtypes

import concourse.bass as bass
import concourse.tile as tile
from concourse import bacc, mybir
from concourse.bass_utils import run_bass_kernel_spmd

BF16 = mybir.dt.bfloat16
F32 = mybir.dt.float32
AF = mybir.ActivationFunctionType
ALU = mybir.AluOpType

B, L, E = 2, 2048, 2048
H, D = 16, 128
NCORES = 1
KT = E // 128              # 16 contraction tiles
LC = L // 512              # 4 column chunks of L
QT = L // 128              # 16 q tiles
EC = E // 512              # 4 output-column chunks
THETA = 10000.0

_PROG = None


def _build_program():
    nc = bacc.Bacc("TRN2", target_bir_lowering=False, debug=False,
                   enable_asserts=False)

    xT_d = nc.dram_tensor("xT", [B, E, L], BF16, kind="ExternalInput").ap()
    w_d = nc.dram_tensor("w", [H, E, 3 * 128], BF16, kind="ExternalInput").ap()
    wo_d = nc.dram_tensor("wo", [H, D, E], BF16, kind="ExternalInput").ap()
    cos_d = nc.dram_tensor("cosf", [128, L], BF16, kind="ExternalInput").ap()
    sin_d = nc.dram_tensor("sinf", [128, L], BF16, kind="ExternalInput").ap()
    tri_d = nc.dram_tensor("tri", [128, 128], F32, kind="ExternalInput").ap()
    aT_d = nc.dram_tensor("aT", [H, 128, L], BF16, kind="Internal").ap()
    y_d = nc.dram_tensor("y", [B, L, E], BF16, kind="ExternalOutput").ap()

    with tile.TileContext(nc) as tc:
        with tc.tile_pool(name="consts", bufs=1) as cpool, \
             tc.tile_pool(name="xres", bufs=1) as xpool, \
             tc.tile_pool(name="wstr", bufs=2) as wpool, \
             tc.tile_pool(name="rope", bufs=4) as rpool, \
             tc.tile_pool(name="qkv", bufs=2) as qkvpool, \
             tc.tile_pool(name="pp", bufs=2) as ppool, \
             tc.tile_pool(name="small", bufs=8) as spool, \
             tc.tile_pool(name="outp", bufs=4) as opool, \
             tc.tile_pool(name="oproj", bufs=2) as projpool, \
             tc.tile_pool(name="ys", bufs=2) as ypool, \
             tc.tile_pool(name="psq", bufs=1, space="PSUM") as psq, \
             tc.tile_pool(name="psa", bufs=3, space="PSUM") as psa, \
             tc.tile_pool(name="pso", bufs=1, space="PSUM") as pso, \
             tc.tile_pool(name="psy", bufs=1, space="PSUM") as psy:

            cosf = cpool.tile([128, L], BF16, tag="cos")
            nc.sync.dma_start(cosf[:], cos_d[:])
            sinf = cpool.tile([128, L], BF16, tag="sin")
            nc.sync.dma_start(sinf[:], sin_d[:])
            tri = cpool.tile([128, 128], F32, tag="tri")
            nc.sync.dma_start(tri[:], tri_d[:])

            for rep in range(int(os.environ.get("KREP", "1"))):
              for b in range(B):
                xsb = xpool.tile([128, KT, L], BF16, tag="xsb")
                # prefetch queue carries loads so they issue during the
                # previous batch's out-projection; chunks let the first
                # QKV matmuls start before the full 8MB lands
                xr = xT_d[b].rearrange("(kt p) l -> p kt l", p=128)
                _xq = getattr(nc, os.environ.get("XQ", "gpsimd"))
                _nch = int(os.environ.get("XCH", "16"))
                _w = KT // _nch
                for xc in range(_nch):
                    _xq.dma_start(xsb[:, _w * xc:_w * (xc + 1), :],
                                  xr[:, _w * xc:_w * (xc + 1), :])

                for h in range(H):
                    wsb = wpool.tile([128, KT, 3 * 128], BF16, tag="wsb")
                    getattr(nc, os.environ.get("WQ", "gpsimd")).dma_start(
                        wsb[:], w_d[h].rearrange("(kt p) c -> p kt c", p=128))

                    # ---- QKV projection + RoPE ----
                    qT = qkvpool.tile([128, L], BF16, tag="qT")
                    kTt = qkvpool.tile([128, L], BF16, tag="kT")
                    vTs = qkvpool.tile([128, L], BF16, tag="vT")
                    for lc in range(LC):
                        ls = lc * 512
                        pA = psq.tile([128, 512], F32, tag="pA")
                        pB = psq.tile([128, 512], F32, tag="pB")
                        pV = psq.tile([128, 512], F32, tag="pV")
                        for kt in range(KT):
                            st = kt == 0
                            sp = kt == KT - 1
                            nc.tensor.matmul(pA[:], wsb[:, kt, 0:128],
                                             xsb[:, kt, ls:ls + 512],
                                             start=st, stop=sp)
                            nc.tensor.matmul(pB[:], wsb[:, kt, 128:256],
                                             xsb[:, kt, ls:ls + 512],
                                             start=st, stop=sp)
                            nc.tensor.matmul(pV[:], wsb[:, kt, 256:384],
                                             xsb[:, kt, ls:ls + 512],
                                             start=st, stop=sp)
                        # RoPE: A = [x1q;x1k], B = [x2q;x2k]
                        t1 = rpool.tile([128, 512], F32, tag="rt")
                        nc.vector.tensor_mul(t1[:], pA[:], cosf[:, ls:ls + 512])
                        t2 = rpool.tile([128, 512], F32, tag="rt")
                        nc.vector.tensor_mul(t2[:], pB[:], sinf[:, ls:ls + 512])
                        et = rpool.tile([128, 512], BF16, tag="ro")
                        nc.gpsimd.tensor_sub(et[:], t1[:], t2[:])
                        t3 = rpool.tile([128, 512], F32, tag="rt")
                        nc.vector.tensor_mul(t3[:], pA[:], sinf[:, ls:ls + 512])
                        t4 = rpool.tile([128, 512], F32, tag="rt")
                        nc.vector.tensor_mul(t4[:], pB[:], cosf[:, ls:ls + 512])
                        ot = rpool.tile([128, 512], BF16, tag="ro")
                        nc.gpsimd.tensor_add(ot[:], t3[:], t4[:])
                        nc.vector.tensor_copy(qT[0:64, ls:ls + 512], et[0:64, :])
                        nc.vector.tensor_copy(qT[64:128, ls:ls + 512], ot[0:64, :])
                        nc.vector.tensor_copy(kTt[0:64, ls:ls + 512], et[64:128, :])
                        nc.vector.tensor_copy(kTt[64:128, ls:ls + 512], ot[64:128, :])
                        nc.scalar.copy(vTs[:, ls:ls + 512], pV[:])
                    v_nat = qkvpool.tile([128, KT, 128], BF16, tag="vn")
                    nc.scalar.dma_start_transpose(out=v_nat[:], in_=vTs[:])

                    # ---- attention ----
                    po = None
                    for i in range(QT):
                        band = (i + 1) * 128
                        nch = (band + 511) // 512
                        pt_t = ppool.tile([128, L], BF16, tag="P")
                        ds = spool.tile([128, 4], F32, tag="ds")
                        for c in range(nch):
                            c0 = c * 512
                            w = min(512, band - c0)
                            s_ps = psa.tile([128, 512], F32, tag="s")
                            nc.tensor.matmul(
                                s_ps[:, 0:w], qT[:, i * 128:(i + 1) * 128],
                                kTt[:, c0:c0 + w], start=True, stop=True)
                            if c == nch - 1:
                                nc.vector.tensor_tensor(
                                    s_ps[:, w - 128:w], s_ps[:, w - 128:w],
                                    tri[:], op=ALU.add)
                            nc.scalar.activation(
                                pt_t[:, c0:c0 + w], s_ps[:, 0:w], AF.Exp,
                                accum_out=ds[:, c:c + 1])
                        dtot = spool.tile([128, 1], F32, tag="dt")
                        if nch > 1:
                            nc.vector.reduce_sum(dtot[:], ds[:, 0:nch],
                                                 axis=mybir.AxisListType.X)
                        else:
                            nc.vector.tensor_copy(dtot[:], ds[:, 0:1])
                        rinv = spool.tile([128, 1], F32, tag="ri")
                        nc.vector.reciprocal(rinv[:], dtot[:])
                        nc.vector.tensor_scalar_mul(pt_t[:, 0:band], pt_t[:, 0:band],
                                                    rinv[:])
                        ptr = ppool.tile([128, KT, 128], BF16, tag="PT")
                        nc.scalar.dma_start_transpose(out=ptr[:, 0:i + 1, :],
                                                      in_=pt_t[:, 0:band])
                        if i % 4 == 0:
                            po = pso.tile([128, 512], F32, tag="po")
                        osl = (i % 4) * 128
                        for kb in range(i + 1):
                            nc.tensor.matmul(
                                po[:, osl:osl + 128], v_nat[:, kb, :],
                                ptr[:, kb, :], start=(kb == 0), stop=(kb == i))
                        if i % 4 == 3:
                            g = i // 4
                            outsb = opool.tile([128, 512], BF16, tag="outT")
                            nc.scalar.copy(outsb[:], po[:])
                            nc.scalar.dma_start(
                                aT_d[h, :, g * 512:(g + 1) * 512], outsb[:])

                # ---- output projection (sums all 16 heads in PSUM) ----
                for ec in range(EC):
                    es = ec * 512
                    wo_ec = projpool.tile([128, H, 512], BF16, tag="wo")
                    getattr(nc, os.environ.get("OQ", "sync")).dma_start(
                        wo_ec[:], wo_d[:, :, es:es + 512].rearrange("h p e -> p h e"))
                    for i in range(QT):
                        asb = projpool.tile([128, H, 128], BF16, tag="asb")
                        getattr(nc, os.environ.get("OQ", "sync")).dma_start(
                            asb[:],
                            aT_d[:, :, i * 128:(i + 1) * 128].rearrange(
                                "h p c -> p h c"))
                        yp = psy.tile([128, 512], F32, tag="psy")
                        for hh in range(H):
                            nc.tensor.matmul(
                                yp[:], asb[:, hh, :], wo_ec[:, hh, :],
                                start=(hh == 0), stop=(hh == H - 1))
                        ysb = ypool.tile([128, 512], BF16, tag="ysb")
                        nc.scalar.copy(ysb[:], yp[:])
                        nc.scalar.dma_start(
                            y_d[b, i * 128:(i + 1) * 128, es:es + 512], ysb[:])

    nc.compile()
    return nc


def _get_program():
    global _PROG
    if _PROG is None:
        _PROG = _build_program()
    return _PROG


def make_in_maps(x, Wq, Wk, Wv, Wo):
    """Host-side layout prep. Returns list of NCORES (=1) input maps."""
    bf = ml_dtypes.bfloat16
    x = np.asarray(x, np.float32)
    Wq = np.asarray(Wq, np.float32)
    Wk = np.asarray(Wk, np.float32)
    Wv = np.asarray(Wv, np.float32)
    Wo = np.asarray(Wo, np.float32)

    xT = np.ascontiguousarray(x.transpose(0, 2, 1)).astype(bf)  # [B, E, L]

    inv = THETA ** (-np.arange(0, D, 2, dtype=np.float32) / D)  # [64]
    ang = np.arange(L, dtype=np.float32)[:, None] * inv[None, :]  # [L, 64]
    cosf = np.ascontiguousarray(np.concatenate([np.cos(ang).T] * 2, axis=0)
                                ).astype(bf)  # [128, L]
    sinf = np.ascontiguousarray(np.concatenate([np.sin(ang).T] * 2, axis=0)
                                ).astype(bf)
    r = np.arange(128)
    tri = np.where(r[None, :] <= r[:, None], 0.0, -1e30).astype(np.float32)

    qscale = np.float32(D ** -0.5)
    ev = np.arange(0, D, 2)
    od = np.arange(1, D, 2)

    w_all = np.empty((H, E, 3, 128), np.float32)
    for h in range(H):
        c0 = h * D
        w_all[h, :, 0, 0:64] = Wq[:, c0 + ev] * qscale
        w_all[h, :, 0, 64:128] = Wk[:, c0 + ev]
        w_all[h, :, 1, 0:64] = Wq[:, c0 + od] * qscale
        w_all[h, :, 1, 64:128] = Wk[:, c0 + od]
        w_all[h, :, 2, :] = Wv[:, c0:c0 + D]

    return [{
        "xT": xT,
        "w": np.ascontiguousarray(w_all.reshape(H, E, 3 * 128)).astype(bf),
        "wo": np.ascontiguousarray(Wo.reshape(H, D, E)).astype(bf),
        "cosf": cosf,
        "sinf": sinf,
        "tri": tri,
    }]


def kernel(x, Wq, Wk, Wv, Wo, bo):
    nc = _get_program()
    maps = make_in_maps(x, Wq, Wk, Wv, Wo)
    res = run_bass_kernel_spmd(nc, maps, core_ids=list(range(NCORES)))
    y = np.asarray(res.results[0]["y"], np.float64)
    y = y + np.asarray(bo, np.float64)[None, None, :]
    return y.astype(np.float32)


# revision 25
# speedup vs baseline: 9.6197x; 2.9168x over previous
"""Causal self-attention (B=2, L=2048, E=2048, H=16, D=128) on ONE trn2 core.

Measured bottleneck in this harness is the per-call axon/PJRT dispatch path,
not device compute: with an 8-core mesh every call re-shards ~300MB of
replicated inputs/partial outputs through the tunnel (~25ms/iter), while a
single-core program keeps all buffers device-resident (bytes are free) and
pays only the fixed dispatch latency. So: one core, whole problem, minimal
I/O, then minimize device exec time (it adds ~1:1 to steady-state wall).

Device kernel per batch:
  - xT[b] resident in SBUF [128, KT, L] bf16; per-head weight slices streamed
    (stationary operand), so QKV needs no x re-DMA per head.
  - per head: qT/kT/vT [D=128, L] via matmul; host pre-permutes Wq/Wk columns
    to (evens, odds) so RoPE is 6 contiguous [128,512] elementwise ops.
  - scores per 128-row q-tile over the causal band, fp32 PSUM; additive -1e30
    triangular mask on the diagonal block; Exp on ACT with accum_out row sums;
    P normalized by reciprocal(rowsum); transposed with one xbar DMA;
    attn@v accumulates in PSUM; per-head output tiles parked in DRAM scratch.
  - out-projection: ec-outer over 512-wide E slices, Wo slice resident,
    attention outputs streamed back, PSUM accumulates over all 16 heads.

Engines execute their instruction queues IN ORDER, so emission order is the
schedule: scores(i+1) is emitted before attn@v(i) (which waits on the
exp/normalize/transpose chain), and the next head's QKV matmuls are emitted
interleaved into the current head's attention slots so the PE fills chain
stalls with projection work. The next batch's first QKV rides inside the
out-projection the same way. Host adds bo and casts to f32.
"""

import os

import numpy as np
import ml_dtypes

import concourse.bass as bass
import concourse.tile as tile
from concourse import bacc, mybir
from concourse.bass_utils import run_bass_kernel_spmd

BF16 = mybir.dt.bfloat16
F32 = mybir.dt.float32
AF = mybir.ActivationFunctionType
ALU = mybir.AluOpType

B, L, E = 2, 2048, 2048
H, D = 16, 128
NCORES = 1
KT = E // 128              # 16 contraction tiles
LC = L // 512              # 4 column chunks of L
QT = L // 128              # 16 q tiles
EC = E // 512              # 4 output-column chunks
THETA = 10000.0

_PROG = None


def _build_program():
    nc = bacc.Bacc("TRN2", target_bir_lowering=False, debug=False,
                   enable_asserts=False)

    xT_d = nc.dram_tensor("xT", [B, E, L], BF16, kind="ExternalInput").ap()
    w_d = nc.dram_tensor("w", [H, E, 3 * 128], BF16, kind="ExternalInput").ap()
    wo_d = nc.dram_tensor("wo", [H, D, E], BF16, kind="ExternalInput").ap()
    cos_d = nc.dram_tensor("cosf", [128, L], BF16, kind="ExternalInput").ap()
    sin_d = nc.dram_tensor("sinf", [128, L], BF16, kind="ExternalInput").ap()
    tri_d = nc.dram_tensor("tri", [128, 128], F32, kind="ExternalInput").ap()
    aT_d = nc.dram_tensor("aT", [H, 128, L], BF16, kind="Internal").ap()
    y_d = nc.dram_tensor("y", [B, L, E], BF16, kind="ExternalOutput").ap()

    with tile.TileContext(nc) as tc:
        with tc.tile_pool(name="consts", bufs=1) as cpool, \
             tc.tile_pool(name="xres", bufs=1) as xpool, \
             tc.tile_pool(name="wstr", bufs=2) as wpool, \
             tc.tile_pool(name="rope", bufs=4) as rpool, \
             tc.tile_pool(name="qkv", bufs=2) as qkvpool, \
             tc.tile_pool(name="pp", bufs=2) as ppool, \
             tc.tile_pool(name="small", bufs=8) as spool, \
             tc.tile_pool(name="outp", bufs=4) as opool, \
             tc.tile_pool(name="oproj", bufs=2) as projpool, \
             tc.tile_pool(name="ys", bufs=2) as ypool, \
             tc.tile_pool(name="psq", bufs=1, space="PSUM") as psq, \
             tc.tile_pool(name="psa", bufs=3, space="PSUM") as psa, \
             tc.tile_pool(name="pso", bufs=1, space="PSUM") as pso, \
             tc.tile_pool(name="psy", bufs=1, space="PSUM") as psy:

            cosf = cpool.tile([128, L], BF16, tag="cos")
            nc.sync.dma_start(cosf[:], cos_d[:])
            sinf = cpool.tile([128, L], BF16, tag="sin")
            nc.sync.dma_start(sinf[:], sin_d[:])
            tri = cpool.tile([128, 128], F32, tag="tri")
            nc.sync.dma_start(tri[:], tri_d[:])

            def emit_x_prefetch(b):
                xsb = xpool.tile([128, KT, L], BF16, tag="xsb",
                                 name=f"xsb_b{b}")
                xr = xT_d[b].rearrange("(kt p) l -> p kt l", p=128)
                for kt in range(KT):
                    nc.gpsimd.dma_start(xsb[:, kt, :], xr[:, kt, :])
                return xsb

            def start_qkv(b, h, xsb, wsb=None):
                if wsb is None:
                    wsb = wpool.tile([128, KT, 3 * 128], BF16, tag="wsb",
                                     name=f"wsb_b{b}h{h}")
                    nc.gpsimd.dma_start(
                        wsb[:], w_d[h].rearrange("(kt p) c -> p kt c", p=128))
                qT = qkvpool.tile([128, L], BF16, tag="qT", name=f"qT_b{b}h{h}")
                kTt = qkvpool.tile([128, L], BF16, tag="kT", name=f"kT_b{b}h{h}")
                vTs = qkvpool.tile([128, L], BF16, tag="vT", name=f"vT_b{b}h{h}")
                return {"wsb": wsb, "qT": qT, "kT": kTt, "vT": vTs,
                        "v_nat": None, "ps": None, "xsb": xsb, "chain": {},
                        "po": None}

            def emit_qkv_step(b, h, st, step):
                # step 0..15: (lc = step//4, kt quad = step%4); rope at quad 3
                lc, quad = divmod(step, 4)
                ls = lc * 512
                wsb, xsb = st["wsb"], st["xsb"]
                if quad == 0:
                    pA = psq.tile([128, 512], F32, tag="pA")
                    pB = psq.tile([128, 512], F32, tag="pB")
                    pV = psq.tile([128, 512], F32, tag="pV")
                    st["ps"] = (pA, pB, pV)
                pA, pB, pV = st["ps"]
                for kt in range(quad * 4, quad * 4 + 4):
                    stt = kt == 0
                    stp = kt == KT - 1
                    nc.tensor.matmul(pA[:], wsb[:, kt, 0:128],
                                     xsb[:, kt, ls:ls + 512],
                                     start=stt, stop=stp)
                    nc.tensor.matmul(pB[:], wsb[:, kt, 128:256],
                                     xsb[:, kt, ls:ls + 512],
                                     start=stt, stop=stp)
                    nc.tensor.matmul(pV[:], wsb[:, kt, 256:384],
                                     xsb[:, kt, ls:ls + 512],
                                     start=stt, stop=stp)
                if quad == 3:
                    qT, kTt, vTs = st["qT"], st["kT"], st["vT"]
                    # RoPE: A = [x1q;x1k], B = [x2q;x2k]
                    t1 = rpool.tile([128, 512], F32, tag="rt")
                    nc.vector.tensor_mul(t1[:], pA[:], cosf[:, ls:ls + 512])
                    t2 = rpool.tile([128, 512], F32, tag="rt")
                    nc.vector.tensor_mul(t2[:], pB[:], sinf[:, ls:ls + 512])
                    et = rpool.tile([128, 512], BF16, tag="ro")
                    nc.gpsimd.tensor_sub(et[:], t1[:], t2[:])
                    t3 = rpool.tile([128, 512], F32, tag="rt")
                    nc.vector.tensor_mul(t3[:], pA[:], sinf[:, ls:ls + 512])
                    t4 = rpool.tile([128, 512], F32, tag="rt")
                    nc.vector.tensor_mul(t4[:], pB[:], cosf[:, ls:ls + 512])
                    ot = rpool.tile([128, 512], BF16, tag="ro")
                    nc.gpsimd.tensor_add(ot[:], t3[:], t4[:])
                    nc.vector.tensor_copy(qT[0:64, ls:ls + 512], et[0:64, :])
                    nc.vector.tensor_copy(qT[64:128, ls:ls + 512], ot[0:64, :])
                    nc.vector.tensor_copy(kTt[0:64, ls:ls + 512], et[64:128, :])
                    nc.vector.tensor_copy(kTt[64:128, ls:ls + 512],
                                          ot[64:128, :])
                    nc.scalar.copy(vTs[:, ls:ls + 512], pV[:])
                    if lc == LC - 1:
                        v_nat = qkvpool.tile([128, KT, 128], BF16, tag="vn",
                                             name=f"vn_b{b}h{h}")
                        nc.scalar.dma_start_transpose(out=v_nat[:], in_=vTs[:])
                        st["v_nat"] = v_nat

            def emit_scores(b, h, st, i):
                band = (i + 1) * 128
                nch = (band + 511) // 512
                qT, kTt = st["qT"], st["kT"]
                pt_t = ppool.tile([128, L], BF16, tag="P", name=f"P_{i}")
                ds = spool.tile([128, 4], F32, tag="ds")
                for c in range(nch):
                    c0 = c * 512
                    w = min(512, band - c0)
                    s_ps = psa.tile([128, 512], F32, tag="s")
                    nc.tensor.matmul(
                        s_ps[:, 0:w], qT[:, i * 128:(i + 1) * 128],
                        kTt[:, c0:c0 + w], start=True, stop=True)
                    if c == nch - 1:
                        nc.vector.tensor_tensor(
                            s_ps[:, w - 128:w], s_ps[:, w - 128:w],
                            tri[:], op=ALU.add)
                    nc.scalar.activation(
                        pt_t[:, c0:c0 + w], s_ps[:, 0:w], AF.Exp,
                        accum_out=ds[:, c:c + 1])
                dtot = spool.tile([128, 1], F32, tag="dt")
                if nch > 1:
                    nc.vector.reduce_sum(dtot[:], ds[:, 0:nch],
                                         axis=mybir.AxisListType.X)
                else:
                    nc.vector.tensor_copy(dtot[:], ds[:, 0:1])
                rinv = spool.tile([128, 1], F32, tag="ri")
                nc.vector.reciprocal(rinv[:], dtot[:])
                nc.vector.tensor_scalar_mul(pt_t[:, 0:band], pt_t[:, 0:band],
                                            rinv[:])
                ptr = ppool.tile([128, KT, 128], BF16, tag="PT", name=f"PT_{i}")
                nc.scalar.dma_start_transpose(out=ptr[:, 0:i + 1, :],
                                              in_=pt_t[:, 0:band])
                st["chain"][i] = ptr

            def emit_attnv(b, h, st, i):
                ptr = st["chain"].pop(i)
                v_nat = st["v_nat"]
                if i % 4 == 0:
                    st["po"] = pso.tile([128, 512], F32, tag="po", name="po")
                po = st["po"]
                osl = (i % 4) * 128
                for kb in range(i + 1):
                    nc.tensor.matmul(
                        po[:, osl:osl + 128], v_nat[:, kb, :],
                        ptr[:, kb, :], start=(kb == 0), stop=(kb == i))
                if i % 4 == 3:
                    g = i // 4
                    outsb = opool.tile([128, 512], BF16, tag="outT")
                    nc.scalar.copy(outsb[:], po[:])
                    nc.scalar.dma_start(
                        aT_d[h, :, g * 512:(g + 1) * 512], outsb[:])

            for rep in range(int(os.environ.get("KREP", "1"))):
              states = {}
              wsb0 = wpool.tile([128, KT, 3 * 128], BF16, tag="wsb",
                                name=f"wsb_first_{rep}")
              nc.gpsimd.dma_start(
                  wsb0[:], w_d[0].rearrange("(kt p) c -> p kt c", p=128))
              xsb = emit_x_prefetch(0)
              states[(0, 0)] = start_qkv(0, 0, xsb, wsb=wsb0)
              for step in range(16):
                  emit_qkv_step(0, 0, states[(0, 0)], step)
              for b in range(B):
                for h in range(H):
                    st = states.pop((b, h))
                    nxt = None
                    if h + 1 < H:
                        nxt = start_qkv(b, h + 1, xsb)
                        states[(b, h + 1)] = nxt
                    for i in range(QT):
                        emit_scores(b, h, st, i)
                        emit_attnv(b, h, st, i)
                    if nxt is not None:
                        for step in range(16):
                            emit_qkv_step(b, h + 1, nxt, step)

                # ---- output projection (sums all 16 heads in PSUM) ----
                if b + 1 < B:
                    xsb = emit_x_prefetch(b + 1)
                    states[(b + 1, 0)] = start_qkv(b + 1, 0, xsb)
                for ec in range(EC):
                    es = ec * 512
                    wo_ec = projpool.tile([128, H, 512], BF16, tag="wo")
                    nc.sync.dma_start(
                        wo_ec[:],
                        wo_d[:, :, es:es + 512].rearrange("h p e -> p h e"))
                    for i in range(QT):
                        asb = projpool.tile([128, H, 128], BF16, tag="asb")
                        nc.sync.dma_start(
                            asb[:],
                            aT_d[:, :, i * 128:(i + 1) * 128].rearrange(
                                "h p c -> p h c"))
                        yp = psy.tile([128, 512], F32, tag="psy")
                        for hh in range(H):
                            nc.tensor.matmul(
                                yp[:], asb[:, hh, :], wo_ec[:, hh, :],
                                start=(hh == 0), stop=(hh == H - 1))
                        ysb = ypool.tile([128, 512], BF16, tag="ysb")
                        nc.scalar.copy(ysb[:], yp[:])
                        nc.scalar.dma_start(
                            y_d[b, i * 128:(i + 1) * 128, es:es + 512], ysb[:])
                if b + 1 < B:
                    for step in range(16):
                        emit_qkv_step(b + 1, 0, states[(b + 1, 0)], step)

    nc.compile()
    return nc


def _get_program():
    global _PROG
    if _PROG is None:
        _PROG = _build_program()
    return _PROG


def make_in_maps(x, Wq, Wk, Wv, Wo):
    """Host-side layout prep. Returns list of NCORES (=1) input maps."""
    bf = ml_dtypes.bfloat16
    x = np.asarray(x, np.float32)
    Wq = np.asarray(Wq, np.float32)
    Wk = np.asarray(Wk, np.float32)
    Wv = np.asarray(Wv, np.float32)
    Wo = np.asarray(Wo, np.float32)

    xT = np.ascontiguousarray(x.transpose(0, 2, 1)).astype(bf)  # [B, E, L]

    inv = THETA ** (-np.arange(0, D, 2, dtype=np.float32) / D)  # [64]
    ang = np.arange(L, dtype=np.float32)[:, None] * inv[None, :]  # [L, 64]
    cosf = np.ascontiguousarray(np.concatenate([np.cos(ang).T] * 2, axis=0)
                                ).astype(bf)  # [128, L]
    sinf = np.ascontiguousarray(np.concatenate([np.sin(ang).T] * 2, axis=0)
                                ).astype(bf)
    r = np.arange(128)
    tri = np.where(r[None, :] <= r[:, None], 0.0, -1e30).astype(np.float32)

    qscale = np.float32(D ** -0.5)
    ev = np.arange(0, D, 2)
    od = np.arange(1, D, 2)

    w_all = np.empty((H, E, 3, 128), np.float32)
    for h in range(H):
        c0 = h * D
        w_all[h, :, 0, 0:64] = Wq[:, c0 + ev] * qscale
        w_all[h, :, 0, 64:128] = Wk[:, c0 + ev]
        w_all[h, :, 1, 0:64] = Wq[:, c0 + od] * qscale
        w_all[h, :, 1, 64:128] = Wk[:, c0 + od]
        w_all[h, :, 2, :] = Wv[:, c0:c0 + D]

    return [{
        "xT": xT,
        "w": np.ascontiguousarray(w_all.reshape(H, E, 3 * 128)).astype(bf),
        "wo": np.ascontiguousarray(Wo.reshape(H, D, E)).astype(bf),
        "cosf": cosf,
        "sinf": sinf,
        "tri": tri,
    }]


def kernel(x, Wq, Wk, Wv, Wo, bo):
    nc = _get_program()
    maps = make_in_maps(x, Wq, Wk, Wv, Wo)
    res = run_bass_kernel_spmd(nc, maps, core_ids=list(range(NCORES)))
    y = np.asarray(res.results[0]["y"], np.float64)
    y = y + np.asarray(bo, np.float64)[None, None, :]
    return y.astype(np.float32)


# revision 29
# speedup vs baseline: 10.7592x; 1.1185x over previous
"""Causal self-attention (B=2, L=2048, E=2048, H=16, D=128) on ONE trn2 core.

Measured bottleneck in this harness is the per-call axon/PJRT dispatch path,
not device compute: with an 8-core mesh every call re-shards ~300MB of
replicated inputs/partial outputs through the tunnel (~25ms/iter), while a
single-core program keeps all buffers device-resident (bytes are free) and
pays only the fixed dispatch latency. So: one core, whole problem, minimal
I/O, then minimize device exec time (it adds ~1:1 to steady-state wall).

Device kernel per batch:
  - xT[b] resident in SBUF [128, KT, L] bf16; per-head weight slices streamed
    (stationary operand), so QKV needs no x re-DMA per head.
  - per head: qT/kT/vT [D=128, L] via matmul; host pre-permutes Wq/Wk columns
    to (evens, odds) so RoPE is 6 contiguous [128,512] elementwise ops.
  - scores per 128-row q-tile over the causal band, fp32 PSUM; additive -1e30
    triangular mask on the diagonal block; Exp on ACT with accum_out row sums;
    P normalized by reciprocal(rowsum); transposed with one xbar DMA;
    attn@v accumulates in PSUM; per-head output tiles parked in DRAM scratch.
  - out-projection: ec-outer over 512-wide E slices, Wo slice resident,
    attention outputs streamed back, PSUM accumulates over all 16 heads.

Engines execute their instruction queues IN ORDER, so emission order is the
schedule: scores(i+1) is emitted before attn@v(i) (which waits on the
exp/normalize/transpose chain), and the next head's QKV matmuls are emitted
interleaved into the current head's attention slots so the PE fills chain
stalls with projection work. The next batch's first QKV rides inside the
out-projection the same way. Host adds bo and casts to f32.
"""

import os

import numpy as np
import ml_dtypes

import concourse.bass as bass
import concourse.tile as tile
from concourse import bacc, mybir
from concourse.bass_utils import run_bass_kernel_spmd

BF16 = mybir.dt.bfloat16
F32 = mybir.dt.float32
AF = mybir.ActivationFunctionType
ALU = mybir.AluOpType

B, L, E = 2, 2048, 2048
H, D = 16, 128
NCORES = 1
KT = E // 128              # 16 contraction tiles
LC = L // 512              # 4 column chunks of L
QT = L // 128              # 16 q tiles
EC = E // 512              # 4 output-column chunks
THETA = 10000.0

_PROG = None


def _build_program():
    nc = bacc.Bacc("TRN2", target_bir_lowering=False, debug=False,
                   enable_asserts=False)

    xT_d = nc.dram_tensor("xT", [B, E, L], BF16, kind="ExternalInput").ap()
    w_d = nc.dram_tensor("w", [H, E, 3 * 128], BF16, kind="ExternalInput").ap()
    wo_d = nc.dram_tensor("wo", [H, D, E], BF16, kind="ExternalInput").ap()
    cos_d = nc.dram_tensor("cosf", [128, L], BF16, kind="ExternalInput").ap()
    sin_d = nc.dram_tensor("sinf", [128, L], BF16, kind="ExternalInput").ap()
    tri_d = nc.dram_tensor("tri", [128, 128], F32, kind="ExternalInput").ap()
    aT_d = nc.dram_tensor("aT", [H, 128, L], BF16, kind="Internal").ap()
    y_d = nc.dram_tensor("y", [B, L, E], BF16, kind="ExternalOutput").ap()

    with tile.TileContext(nc) as tc:
        with tc.tile_pool(name="consts", bufs=1) as cpool, \
             tc.tile_pool(name="xres", bufs=1) as xpool, \
             tc.tile_pool(name="wstr", bufs=2) as wpool, \
             tc.tile_pool(name="rope", bufs=4) as rpool, \
             tc.tile_pool(name="qkv", bufs=2) as qkvpool, \
             tc.tile_pool(name="pp", bufs=2) as ppool, \
             tc.tile_pool(name="small", bufs=8) as spool, \
             tc.tile_pool(name="outp", bufs=4) as opool, \
             tc.tile_pool(name="oproj", bufs=2) as projpool, \
             tc.tile_pool(name="ys", bufs=2) as ypool, \
             tc.tile_pool(name="psq", bufs=1, space="PSUM") as psq, \
             tc.tile_pool(name="psa", bufs=3, space="PSUM") as psa, \
             tc.tile_pool(name="pso", bufs=1, space="PSUM") as pso, \
             tc.tile_pool(name="psy", bufs=1, space="PSUM") as psy:

            cosf = cpool.tile([128, L], BF16, tag="cos")
            nc.sync.dma_start(cosf[:], cos_d[:])
            sinf = cpool.tile([128, L], BF16, tag="sin")
            nc.sync.dma_start(sinf[:], sin_d[:])
            tri = cpool.tile([128, 128], F32, tag="tri")
            nc.sync.dma_start(tri[:], tri_d[:])

            def emit_x_prefetch(b):
                xsb = xpool.tile([128, KT, L], BF16, tag="xsb",
                                 name=f"xsb_b{b}")
                xr = xT_d[b].rearrange("(kt p) l -> p kt l", p=128)
                for kt in range(KT):
                    nc.gpsimd.dma_start(xsb[:, kt, :], xr[:, kt, :])
                return xsb

            def start_qkv(b, h, xsb, wsb=None):
                if wsb is None:
                    wsb = wpool.tile([128, KT, 3 * 128], BF16, tag="wsb",
                                     name=f"wsb_b{b}h{h}")
                    nc.gpsimd.dma_start(
                        wsb[:], w_d[h].rearrange("(kt p) c -> p kt c", p=128))
                qT = qkvpool.tile([128, L], BF16, tag="qT", name=f"qT_b{b}h{h}")
                kTt = qkvpool.tile([128, L], BF16, tag="kT", name=f"kT_b{b}h{h}")
                vTs = qkvpool.tile([128, L], BF16, tag="vT", name=f"vT_b{b}h{h}")
                return {"wsb": wsb, "qT": qT, "kT": kTt, "vT": vTs,
                        "v_nat": None, "ps": None, "xsb": xsb, "chain": {},
                        "po": None}

            def emit_qkv_step(b, h, st, step):
                # step 0..15: (lc = step//4, kt quad = step%4); rope at quad 3
                lc, quad = divmod(step, 4)
                ls = lc * 512
                wsb, xsb = st["wsb"], st["xsb"]
                if quad == 0:
                    pA = psq.tile([128, 512], F32, tag="pA")
                    pB = psq.tile([128, 512], F32, tag="pB")
                    pV = psq.tile([128, 512], F32, tag="pV")
                    st["ps"] = (pA, pB, pV)
                pA, pB, pV = st["ps"]
                for kt in range(quad * 4, quad * 4 + 4):
                    stt = kt == 0
                    stp = kt == KT - 1
                    nc.tensor.matmul(pA[:], wsb[:, kt, 0:128],
                                     xsb[:, kt, ls:ls + 512],
                                     start=stt, stop=stp)
                    nc.tensor.matmul(pB[:], wsb[:, kt, 128:256],
                                     xsb[:, kt, ls:ls + 512],
                                     start=stt, stop=stp)
                    nc.tensor.matmul(pV[:], wsb[:, kt, 256:384],
                                     xsb[:, kt, ls:ls + 512],
                                     start=stt, stop=stp)
                if quad == 3:
                    qT, kTt, vTs = st["qT"], st["kT"], st["vT"]
                    # RoPE: A = [x1q;x1k], B = [x2q;x2k]
                    t1 = rpool.tile([128, 512], F32, tag="rt")
                    nc.vector.tensor_mul(t1[:], pA[:], cosf[:, ls:ls + 512])
                    t2 = rpool.tile([128, 512], F32, tag="rt")
                    nc.vector.tensor_mul(t2[:], pB[:], sinf[:, ls:ls + 512])
                    et = rpool.tile([128, 512], BF16, tag="ro")
                    nc.gpsimd.tensor_sub(et[:], t1[:], t2[:])
                    t3 = rpool.tile([128, 512], F32, tag="rt")
                    nc.vector.tensor_mul(t3[:], pA[:], sinf[:, ls:ls + 512])
                    t4 = rpool.tile([128, 512], F32, tag="rt")
                    nc.vector.tensor_mul(t4[:], pB[:], cosf[:, ls:ls + 512])
                    ot = rpool.tile([128, 512], BF16, tag="ro")
                    nc.gpsimd.tensor_add(ot[:], t3[:], t4[:])
                    nc.vector.tensor_copy(qT[0:64, ls:ls + 512], et[0:64, :])
                    nc.vector.tensor_copy(qT[64:128, ls:ls + 512], ot[0:64, :])
                    nc.vector.tensor_copy(kTt[0:64, ls:ls + 512], et[64:128, :])
                    nc.vector.tensor_copy(kTt[64:128, ls:ls + 512],
                                          ot[64:128, :])
                    nc.scalar.copy(vTs[:, ls:ls + 512], pV[:])
                    if lc == LC - 1:
                        v_nat = qkvpool.tile([128, KT, 128], BF16, tag="vn",
                                             name=f"vn_b{b}h{h}")
                        nc.scalar.dma_start_transpose(out=v_nat[:], in_=vTs[:])
                        st["v_nat"] = v_nat

            def emit_scores(b, h, st, i):
                band = (i + 1) * 128
                nch = (band + 511) // 512
                qT, kTt = st["qT"], st["kT"]
                pt_t = ppool.tile([128, L], BF16, tag="P", name=f"P_{i}")
                ds = spool.tile([128, 4], F32, tag="ds")
                for c in range(nch):
                    c0 = c * 512
                    w = min(512, band - c0)
                    s_ps = psa.tile([128, 512], F32, tag="s")
                    nc.tensor.matmul(
                        s_ps[:, 0:w], qT[:, i * 128:(i + 1) * 128],
                        kTt[:, c0:c0 + w], start=True, stop=True)
                    if c == nch - 1:
                        nc.vector.tensor_tensor(
                            s_ps[:, w - 128:w], s_ps[:, w - 128:w],
                            tri[:], op=ALU.add)
                    nc.scalar.activation(
                        pt_t[:, c0:c0 + w], s_ps[:, 0:w], AF.Exp,
                        accum_out=ds[:, c:c + 1])
                dtot = spool.tile([128, 1], F32, tag="dt")
                if nch > 1:
                    nc.vector.reduce_sum(dtot[:], ds[:, 0:nch],
                                         axis=mybir.AxisListType.X)
                else:
                    nc.vector.tensor_copy(dtot[:], ds[:, 0:1])
                rinv = spool.tile([128, 1], F32, tag="ri")
                nc.vector.reciprocal(rinv[:], dtot[:])
                nc.vector.tensor_scalar_mul(pt_t[:, 0:band], pt_t[:, 0:band],
                                            rinv[:])
                ptr = ppool.tile([128, KT, 128], BF16, tag="PT", name=f"PT_{i}")
                nc.scalar.dma_start_transpose(out=ptr[:, 0:i + 1, :],
                                              in_=pt_t[:, 0:band])
                st["chain"][i] = ptr

            def emit_attnv(b, h, st, i):
                ptr = st["chain"].pop(i)
                v_nat = st["v_nat"]
                if i % 4 == 0:
                    st["po"] = pso.tile([128, 512], F32, tag="po", name="po")
                po = st["po"]
                osl = (i % 4) * 128
                for kb in range(i + 1):
                    nc.tensor.matmul(
                        po[:, osl:osl + 128], v_nat[:, kb, :],
                        ptr[:, kb, :], start=(kb == 0), stop=(kb == i))
                if i % 4 == 3:
                    g = i // 4
                    outsb = opool.tile([128, 512], BF16, tag="outT")
                    nc.scalar.copy(outsb[:], po[:])
                    nc.scalar.dma_start(
                        aT_d[h, :, g * 512:(g + 1) * 512], outsb[:])

            for rep in range(int(os.environ.get("KREP", "1"))):
              states = {}
              wsb0 = wpool.tile([128, KT, 3 * 128], BF16, tag="wsb",
                                name=f"wsb_first_{rep}")
              nc.gpsimd.dma_start(
                  wsb0[:], w_d[0].rearrange("(kt p) c -> p kt c", p=128))
              xsb = emit_x_prefetch(0)
              states[(0, 0)] = start_qkv(0, 0, xsb, wsb=wsb0)
              for step in range(16):
                  emit_qkv_step(0, 0, states[(0, 0)], step)
              for b in range(B):
                for h in range(H):
                    st = states.pop((b, h))
                    nxt = None
                    if h + 1 < H:
                        nxt = start_qkv(b, h + 1, xsb)
                        states[(b, h + 1)] = nxt
                    for i in range(QT):
                        emit_scores(b, h, st, i)
                        emit_attnv(b, h, st, i)
                    if nxt is not None:
                        for step in range(16):
                            emit_qkv_step(b, h + 1, nxt, step)

                # ---- output projection (sums all 16 heads in PSUM) ----
                if b + 1 < B:
                    xsb = emit_x_prefetch(b + 1)
                    states[(b + 1, 0)] = start_qkv(b + 1, 0, xsb)
                for ec in range(EC):
                    es = ec * 512
                    wo_ec = projpool.tile([128, H, 512], BF16, tag="wo")
                    nc.sync.dma_start(
                        wo_ec[:],
                        wo_d[:, :, es:es + 512].rearrange("h p e -> p h e"))
                    for i in range(QT):
                        asb = projpool.tile([128, H, 128], BF16, tag="asb")
                        nc.sync.dma_start(
                            asb[:],
                            aT_d[:, :, i * 128:(i + 1) * 128].rearrange(
                                "h p c -> p h c"))
                        yp = psy.tile([128, 512], F32, tag="psy")
                        for hh in range(H):
                            nc.tensor.matmul(
                                yp[:], asb[:, hh, :], wo_ec[:, hh, :],
                                start=(hh == 0), stop=(hh == H - 1))
                        ysb = ypool.tile([128, 512], BF16, tag="ysb")
                        nc.scalar.copy(ysb[:], yp[:])
                        nc.scalar.dma_start(
                            y_d[b, i * 128:(i + 1) * 128, es:es + 512], ysb[:])
                if b + 1 < B:
                    for step in range(16):
                        emit_qkv_step(b + 1, 0, states[(b + 1, 0)], step)

    nc.compile()
    return nc


def _get_program():
    global _PROG
    if _PROG is None:
        _PROG = _build_program()
    return _PROG


def make_in_maps(x, Wq, Wk, Wv, Wo):
    """Host-side layout prep. Returns list of NCORES (=1) input maps."""
    bf = ml_dtypes.bfloat16
    x = np.asarray(x, np.float32)
    Wq = np.asarray(Wq, np.float32)
    Wk = np.asarray(Wk, np.float32)
    Wv = np.asarray(Wv, np.float32)
    Wo = np.asarray(Wo, np.float32)

    xT = np.ascontiguousarray(x.transpose(0, 2, 1)).astype(bf)  # [B, E, L]

    inv = THETA ** (-np.arange(0, D, 2, dtype=np.float32) / D)  # [64]
    ang = np.arange(L, dtype=np.float32)[:, None] * inv[None, :]  # [L, 64]
    cosf = np.ascontiguousarray(np.concatenate([np.cos(ang).T] * 2, axis=0)
                                ).astype(bf)  # [128, L]
    sinf = np.ascontiguousarray(np.concatenate([np.sin(ang).T] * 2, axis=0)
                                ).astype(bf)
    r = np.arange(128)
    tri = np.where(r[None, :] <= r[:, None], 0.0, -1e30).astype(np.float32)

    qscale = np.float32(D ** -0.5)
    ev = np.arange(0, D, 2)
    od = np.arange(1, D, 2)

    w_all = np.empty((H, E, 3, 128), np.float32)
    for h in range(H):
        c0 = h * D
        w_all[h, :, 0, 0:64] = Wq[:, c0 + ev] * qscale
        w_all[h, :, 0, 64:128] = Wk[:, c0 + ev]
        w_all[h, :, 1, 0:64] = Wq[:, c0 + od] * qscale
        w_all[h, :, 1, 64:128] = Wk[:, c0 + od]
        w_all[h, :, 2, :] = Wv[:, c0:c0 + D]

    return [{
        "xT": xT,
        "w": np.ascontiguousarray(w_all.reshape(H, E, 3 * 128)).astype(bf),
        "wo": np.ascontiguousarray(Wo.reshape(H, D, E)).astype(bf),
        "cosf": cosf,
        "sinf": sinf,
        "tri": tri,
    }]


def kernel(x, Wq, Wk, Wv, Wo, bo):
    nc = _get_program()
    maps = make_in_maps(x, Wq, Wk, Wv, Wo)
    res = run_bass_kernel_spmd(nc, maps, core_ids=list(range(NCORES)))
    y = np.asarray(res.results[0]["y"], np.float64)
    y = y + np.asarray(bo, np.float64)[None, None, :]
    return y.astype(np.float32)
